# revision 64
# baseline (speedup 1.0000x reference)
"""Trainium2 Bass kernel for nn_MultiHeadAttention (B=2, S=2048, H=1024, 16 heads).

Sharding (Megatron-style tensor parallel over heads):
  - core c owns heads {2c, 2c+1} (hidden dims 128c..128c+127) for BOTH batches.
  - Wq/Wk/Wv row-sharded, Wo column-sharded. Each core emits ONE full-shape
    bf16 partial output (its two heads' contribution to x@Wo.T); the host sums
    the 8 partials and adds bo + bv @ Wo.T.

v17 design (230.6us vs 236us v4):
  - softmax normalized ON DEVICE: dn rows -> 1/dn via integer bit-trick
    seed + one Newton pass (3 standard DVE ops, batched over psum rows
    0..32; the dn bank is pre-zeroed once so garbage rows stay finite) ->
    broadcast across partitions via a bf16 selector matmul -> u = rbc *
    pv (pv parked in SBUF by an ACT copy so its psum bank frees early).
    O-projection becomes a single K=128 matmul per f-chunk; output DMA +
    psum evacuation work is HALVED vs v4 (one partial instead of two).
  - exp split across engines: 12/16 key-chunks use the ACT engine's
    native Exp; 4/16 compute exp on the DVE via the Schraudolph bf16
    bit-trick (int16(s*23.083 + 16250.5) bitcast to bf16), relieving the
    ACT bottleneck (142us of exp in v4).
  - weights shipped pre-rearranged partition-major so their DMAs are one
    contiguous row per partition (descriptor-gen was serializing startup);
    all x tiles prefetched right after the weights.
  - same phase-1 section interleave + score/exp/PV software pipeline as
    v4; O-proj of q-tile N interleaved into chunks 3..13 of q-tile N+1;
    final flush alternates psum banks and ACT/DVE evacuation.
  - PSUM: 2x[128,1024] score ring + pv + dn + o0 + o1 = 8 banks.
"""

import numpy as np

HIDDEN = 1024
HEADS = 16
HD = 64
B, S = 2, 2048
NTOK = B * S            # 4096
NCORES = 8
HSL = HIDDEN // NCORES  # 128 hidden dims per core (2 heads)
P = 128
FCH = HIDDEN // P       # 8 contraction chunks
TOK_TILE = 512
NTT = NTOK // TOK_TILE  # 8 token tiles (4 per batch)
QT_W = 512
NQT = S // QT_W         # 4 q tiles per batch
NKC = S // P            # 16 kv chunks per batch

# Schraudolph bf16 exp: p = bitcast_bf16(int16(s * SCH_C1 + SCH_C2))
# SCH_C1 folds the 1/sqrt(64) attention scale: 0.125 * 128/ln2.
SCH_C1 = 0.125 * 128.0 / float(np.log(2.0))
SCH_C2 = 127.0 * 128.0 - 5.5
# key chunks whose exp runs on DVE (rest on ACT)
DVE_CHUNKS = (3, 7, 11, 14)
# chunks at which the previous q-tile's O-projection steps f=0..7 fire
# (start at 4 so the normalization chain has time to produce u)
OPROJ_CHUNKS = (3, 4, 6, 7, 9, 10, 12, 13)
# Newton-Raphson reciprocal seed: y0 = bitcast_f32(MAGIC1 - bits(x))
NR_MAGIC1 = 0x7EF311C4 + 1

_CACHE = {}


def _build_bass():
    import concourse.bacc as bacc
    import concourse.mybir as mybir
    import concourse.tile as tile

    f32 = mybir.dt.float32
    bf16 = mybir.dt.bfloat16
    i16 = mybir.dt.int16
    i32 = mybir.dt.int32
    Exp = mybir.ActivationFunctionType.Exp
    Ln = mybir.ActivationFunctionType.Ln
    Ident = mybir.ActivationFunctionType.Identity
    Mult = mybir.AluOpType.mult
    Add = mybir.AluOpType.add

    nc = bacc.Bacc("TRN2", target_bir_lowering=False, debug=False,
                   num_devices=NCORES)

    xT = nc.dram_tensor("xT", [HIDDEN, NTOK], bf16, kind="ExternalInput").ap()
    wqT = nc.dram_tensor("wqT", [P, FCH * HSL], bf16,
                         kind="ExternalInput").ap()
    wkT = nc.dram_tensor("wkT", [P, FCH * HSL], bf16,
                         kind="ExternalInput").ap()
    wvT = nc.dram_tensor("wvT", [P, FCH * HSL], bf16,
                         kind="ExternalInput").ap()
    woT = nc.dram_tensor("woT", [HSL, HIDDEN], bf16, kind="ExternalInput").ap()
    bq = nc.dram_tensor("bq", [HSL, 1], f32, kind="ExternalInput").ap()
    bk = nc.dram_tensor("bk", [HSL, 1], f32, kind="ExternalInput").ap()
    onesd = nc.dram_tensor("onesd", [P, 1], bf16, kind="ExternalInput").ap()
    seld = nc.dram_tensor("seld", [33, P], bf16, kind="ExternalInput").ap()
    outT = nc.dram_tensor("outT", [HIDDEN, NTOK], bf16,
                          kind="ExternalOutput").ap()

    with tile.TileContext(nc) as tc:
        import contextlib
        ctx = contextlib.ExitStack()
        with ctx:
            wpool = ctx.enter_context(tc.tile_pool(name="w", bufs=1))
            xpool = ctx.enter_context(tc.tile_pool(name="x", bufs=8))
            qkpool = ctx.enter_context(tc.tile_pool(name="qk", bufs=1))
            vpool = ctx.enter_context(tc.tile_pool(name="v", bufs=1))
            ppool = ctx.enter_context(tc.tile_pool(name="p", bufs=4))
            spool = ctx.enter_context(tc.tile_pool(name="scl", bufs=2))
            apool = ctx.enter_context(tc.tile_pool(name="attn", bufs=2))
            opool = ctx.enter_context(tc.tile_pool(name="osb", bufs=6))
            sps = ctx.enter_context(
                tc.tile_pool(name="sps", bufs=2, space="PSUM"))
            aux = ctx.enter_context(
                tc.tile_pool(name="aux", bufs=1, space="PSUM"))

            # prefetch tile 0's x, then the weights, then the remaining
            # x tiles -- the DMA queue drains in order, and phase-1 needs
            # tile-0 x + weights first; the rest arrive before the
            # interleaved sections touch them
            x_tiles = [xpool.tile([P, FCH, TOK_TILE], bf16, tag="x",
                                  name=f"x{tt}") for tt in range(NTT)]
            x_t0 = x_tiles[0]
            nc.sync.dma_start(
                x_t0, xT[:, 0:TOK_TILE].rearrange("(c p) n -> p c n", p=P))

            # ---- weights / biases ----
            wq_sb = wpool.tile([P, FCH, HSL], bf16)
            wk_sb = wpool.tile([P, FCH, HSL], bf16)
            wv_sb = wpool.tile([P, FCH, HSL], bf16)
            wo_sb = wpool.tile([P, HIDDEN], bf16)
            bq_sb = wpool.tile([P, 1], f32)
            bk_sb = wpool.tile([P, 1], f32)
            ones_sb = wpool.tile([P, 1], bf16)
            sel_sb = wpool.tile([33, P], bf16)
            nc.sync.dma_start(wq_sb, wqT.rearrange("p (c m) -> p c m", c=FCH))
            nc.sync.dma_start(wk_sb, wkT.rearrange("p (c m) -> p c m", c=FCH))
            nc.sync.dma_start(wv_sb, wvT.rearrange("p (c m) -> p c m", c=FCH))
            nc.sync.dma_start(wo_sb, woT)
            nc.sync.dma_start(bq_sb, bq)
            nc.sync.dma_start(bk_sb, bk)
            nc.sync.dma_start(ones_sb, onesd)
            nc.sync.dma_start(sel_sb, seld)

            for tt in range(1, NTT):
                tsl = slice(tt * TOK_TILE, (tt + 1) * TOK_TILE)
                nc.sync.dma_start(
                    x_tiles[tt],
                    xT[:, tsl].rearrange("(c p) n -> p c n", p=P))

            qt_sb = qkpool.tile([P, NTOK], bf16)
            kt_sb = qkpool.tile([P, NTOK], bf16)
            v_sb = vpool.tile([P, B * NKC, HSL], bf16)

            # pre-zero the two "dn" psum ring buffers ONCE: the NR
            # reciprocal runs batched over rows 0..32, and rows 1..31
            # (never written by the M=1 denominator matmuls) must hold 0.0
            # so the seed produces a large-but-finite float there instead
            # of NaN poison for the selector matmul.
            dz = aux.tile([P, QT_W], f32, tag="dn", name="dnz")
            nc.vector.memset(dz[0:33, :], 0.0)

            _sec_ctr = [0]

            def aux_ps(nm):
                # oproj / phase-1 / bcast psums alternate two banks
                tg = "o0" if _sec_ctr[0] % 2 == 0 else "o1"
                _sec_ctr[0] += 1
                return aux.tile([P, QT_W], f32, tag=tg, name=nm)

            def phase1_sections(tt):
                """Return a list of closures, each a ~1-2us slice of the
                projection work for token tile tt; psum comes from the aux
                o0/o1 slots so the score ring stays free."""
                tsl = slice(tt * TOK_TILE, (tt + 1) * TOK_TILE)
                state = {}

                def sec_q():
                    x_t = x_tiles[tt]
                    state["x"] = x_t
                    q_ps = aux_ps(f"qp{tt}")
                    for f in range(FCH):
                        nc.tensor.matmul(q_ps, wq_sb[:, f, :], x_t[:, f, :],
                                         start=(f == 0), stop=(f == FCH - 1))
                    nc.vector.tensor_scalar_add(qt_sb[:, tsl], q_ps, bq_sb)

                def sec_k():
                    x_t = state["x"]
                    k_ps = aux_ps(f"kp{tt}")
                    for f in range(FCH):
                        nc.tensor.matmul(k_ps, wk_sb[:, f, :], x_t[:, f, :],
                                         start=(f == 0), stop=(f == FCH - 1))
                    nc.vector.tensor_scalar_add(kt_sb[:, tsl], k_ps, bk_sb)

                def sec_v(sub):
                    x_t = state["x"]
                    v_ps = aux_ps(f"vp{tt}{sub}")
                    ssl = slice(sub * P, (sub + 1) * P)
                    for f in range(FCH):
                        nc.tensor.matmul(v_ps[:, 0:HSL],
                                         x_t[:, f, ssl], wv_sb[:, f, :],
                                         start=(f == 0), stop=(f == FCH - 1))
                    gc = (tt * TOK_TILE) // P + sub
                    nc.vector.tensor_copy(v_sb[:, gc, :], v_ps[:, 0:HSL])

                return [sec_q, sec_k] + \
                    [(lambda s=s: sec_v(s)) for s in range(TOK_TILE // P)]

            def phase1_tile(tt):
                tsl = slice(tt * TOK_TILE, (tt + 1) * TOK_TILE)
                x_t = x_tiles[tt]

                q_ps = sps.tile([P, TOK_TILE], f32, tag="s", name=f"qp{tt}")
                for f in range(FCH):
                    nc.tensor.matmul(q_ps, wq_sb[:, f, :], x_t[:, f, :],
                                     start=(f == 0), stop=(f == FCH - 1))
                nc.scalar.activation(qt_sb[:, tsl], q_ps, Ident, bias=bq_sb)

                k_ps = sps.tile([P, TOK_TILE], f32, tag="s", name=f"kp{tt}")
                for f in range(FCH):
                    nc.tensor.matmul(k_ps, wk_sb[:, f, :], x_t[:, f, :],
                                     start=(f == 0), stop=(f == FCH - 1))
                nc.scalar.activation(kt_sb[:, tsl], k_ps, Ident, bias=bk_sb)

                # V token-major: out[tok128, 128dims] = x_chunk.T @ wv_chunk
                for sub in range(TOK_TILE // P):
                    v_ps = sps.tile([P, TOK_TILE], f32, tag="s",
                                    name=f"vp{tt}{sub}")
                    ssl = slice(sub * P, (sub + 1) * P)
                    for f in range(FCH):
                        nc.tensor.matmul(v_ps[:, 0:HSL],
                                         x_t[:, f, ssl], wv_sb[:, f, :],
                                         start=(f == 0), stop=(f == FCH - 1))
                    gc = (tt * TOK_TILE) // P + sub
                    nc.vector.tensor_copy(v_sb[:, gc, :], v_ps[:, 0:HSL])

            def oproj_step(b, qt, u_sb, f, final=False):
                qsl = slice(b * S + qt * QT_W, b * S + (qt + 1) * QT_W)
                fsl = slice(f * P, (f + 1) * P)
                if final and f % 2 == 1:
                    # scores are done; borrow ring slots so the 8 flush
                    # steps do not serialize on the single "o" bank
                    o_ps = sps.tile([P, 2 * QT_W], f32, tag="s",
                                    name=f"of{b}{qt}{f}")[:, 0:QT_W]
                else:
                    o_ps = aux_ps(f"o{b}{qt}{f}")
                nc.tensor.matmul(o_ps, wo_sb[:, fsl], u_sb,
                                 start=True, stop=True)
                ob = opool.tile([P, QT_W], bf16, tag="ob",
                                name=f"ob{b}{qt}{f}")
                if final and f % 2 == 0:
                    nc.scalar.copy(ob, o_ps)
                else:
                    nc.vector.tensor_copy(ob, o_ps)
                nc.sync.dma_start(outT[fsl, qsl], ob)

            def phase2_chunks(b, qt, prev, deadlines):
                """scores -> exp -> PV/denominator chunks with the previous
                q-tile's O-projection interleaved; normalize the numerator
                on-device. Returns the normalized u_sb."""
                qsl = slice(b * S + qt * QT_W, b * S + (qt + 1) * QT_W)
                pv01 = aux.tile([P, QT_W], f32, tag="pv", name=f"pv{b}{qt}")
                dn = aux.tile([P, QT_W], f32, tag="dn", name=f"dn{b}{qt}")
                p_tiles = {}

                def emit_scores(c):
                    gc = b * NKC + c
                    ksl = slice(gc * P, (gc + 1) * P)
                    s_c = sps.tile([P, 2 * QT_W], f32, tag="s",
                                   name=f"s{b}{qt}{c}")
                    nc.tensor.matmul(s_c[:, 0:QT_W],
                                     kt_sb[0:HD, ksl], qt_sb[0:HD, qsl],
                                     start=True, stop=True,
                                     tile_position=(0, 0))
                    nc.tensor.matmul(s_c[:, QT_W:2 * QT_W],
                                     kt_sb[HD:P, ksl], qt_sb[HD:P, qsl],
                                     start=True, stop=True,
                                     tile_position=(HD, 0))
                    p_c = ppool.tile([P, 2 * QT_W], bf16, tag="p",
                                     name=f"p{b}{qt}{c}")
                    if c in DVE_CHUNKS:
                        nc.vector.tensor_scalar(
                            p_c.bitcast(i16), s_c, SCH_C1, SCH_C2,
                            op0=Mult, op1=Add)
                    else:
                        nc.scalar.activation(p_c, s_c, Exp, scale=0.125)
                    p_tiles[c] = p_c

                def emit_pv(c):
                    gc = b * NKC + c
                    p_c = p_tiles.pop(c)
                    st = dict(start=(c == 0), stop=(c == NKC - 1),
                              skip_group_check=True)
                    nc.tensor.matmul(pv01[0:HD, :], v_sb[:, gc, 0:HD],
                                     p_c[:, 0:QT_W],
                                     tile_position=(0, 0), **st)
                    nc.tensor.matmul(pv01[HD:P, :], v_sb[:, gc, HD:P],
                                     p_c[:, QT_W:2 * QT_W],
                                     tile_position=(0, HD), **st)
                    nc.tensor.matmul(dn[0:1, :], ones_sb, p_c[:, 0:QT_W],
                                     tile_position=(0, 0), **st)
                    nc.tensor.matmul(dn[32:33, :], ones_sb,
                                     p_c[:, QT_W:2 * QT_W],
                                     tile_position=(0, 32), **st)

                # software-pipelined: scores/exp run one chunk ahead of
                # PV/denominator so exp(c+1) never waits on chunk c's tail
                emit_scores(0)
                emit_scores(1)
                for c in range(NKC):
                    if c + 2 < NKC:
                        while deadlines and deadlines[0][0] <= b * NKC + c + 2:
                            deadlines.pop(0)[1]()
                        emit_scores(c + 2)
                    if prev is not None and c in OPROJ_CHUNKS:
                        oproj_step(prev[0], prev[1], prev[2],
                                   OPROJ_CHUNKS.index(c))
                    if deadlines and c % 2 == 0:
                        deadlines.pop(0)[1]()
                    emit_pv(c)

                # normalize: the two dn row-groups (rows 0/32 even chunks,
                # 64/96 odd chunks) are evacuated on ACT, summed, then 1/dn
                # via bit-trick seed + one Newton pass on the DVE (rows
                # 0..32 batched; rows 1..31 are zeros -> large finite junk
                # killed by the zero selector rows), broadcast over
                # partitions via the bf16 selector matmul, u = pv * rbc
                # pv numerator parks in SBUF via the idle ACT engine so
                # its psum bank frees for the next q-tile immediately,
                # in parallel with the DVE Newton chain
                pvf = spool.tile([P, QT_W], f32, tag="pf", name=f"pf{b}{qt}")
                nc.scalar.copy(pvf, pv01)
                y0 = spool.tile([33, QT_W], f32, tag="y0", name=f"y0{b}{qt}")
                tp = spool.tile([33, QT_W], f32, tag="tp", name=f"tp{b}{qt}")
                rd = spool.tile([33, QT_W], bf16, tag="rd", name=f"rd{b}{qt}")
                nc.vector.tensor_scalar(y0.bitcast(i32),
                                        dn[0:33, :].bitcast(i32),
                                        -1, NR_MAGIC1, op0=Mult, op1=Add)
                nc.vector.scalar_tensor_tensor(tp, dn[0:33, :], -1.0, y0,
                                               op0=Mult, op1=Mult)
                nc.vector.scalar_tensor_tensor(rd, tp, 2.0, y0,
                                               op0=Add, op1=Mult)
                rbc = aux_ps(f"rbc{b}{qt}")
                nc.tensor.matmul(rbc, sel_sb, rd, start=True, stop=True)
                u_sb = apool.tile([P, QT_W], bf16, tag="u", name=f"u{b}{qt}")
                nc.vector.tensor_tensor(u_sb, rbc, pvf, op=Mult)
                return u_sb

            phase1_tile(0)
            deadlines = []
            for tt in range(1, NTT):
                for sec in phase1_sections(tt):
                    deadlines.append((4 * tt, sec))
            prev = None
            for b in range(B):
                for qt in range(NQT):
                    u = phase2_chunks(b, qt, prev, deadlines)
                    prev = (b, qt, u)
            assert not deadlines
            for f in range(FCH):
                oproj_step(prev[0], prev[1], prev[2], f, final=True)

    nc.compile()
    return nc


def _shard_inputs(x, Wq, bq, Wk, bk, Wv, bv, Wo, bo):
    import ml_dtypes
    bf = ml_dtypes.bfloat16

    def _wmajor(w):
        # [HSL, HIDDEN] slice -> transposed [HIDDEN, HSL] -> partition-major
        # [P, FCH*HSL] so the weight DMA is one contiguous row per partition
        wt = w.T.reshape(FCH, P, HSL).transpose(1, 0, 2)
        return np.ascontiguousarray(wt.reshape(P, FCH * HSL)).astype(bf)
    xT = np.ascontiguousarray(
        np.asarray(x).reshape(NTOK, HIDDEN).T).astype(bf)
    ones = np.ones((P, 1), dtype=bf)
    sel = np.zeros((33, P), dtype=bf)
    sel[0, 0:HD] = 1.0
    sel[32, HD:P] = 1.0
    in_maps = []
    for c in range(NCORES):
        rs = slice(HSL * c, HSL * (c + 1))
        in_maps.append({
            "xT": xT,
            "wqT": _wmajor(Wq[rs]),
            "wkT": _wmajor(Wk[rs]),
            "wvT": _wmajor(Wv[rs]),
            "woT": np.ascontiguousarray(Wo[:, rs].T).astype(bf),
            "bq": np.ascontiguousarray(
                bq[rs].reshape(HSL, 1).astype(np.float32)),
            "bk": np.ascontiguousarray(
                bk[rs].reshape(HSL, 1).astype(np.float32)),
            "onesd": ones,
            "seld": sel,
        })
    return in_maps


def kernel(x, Wq, bq, Wk, bk, Wv, bv, Wo, bo):
    from concourse.bass_utils import run_bass_kernel_spmd

    if "nc" not in _CACHE:
        _CACHE["nc"] = _build_bass()
    nc = _CACHE["nc"]

    in_maps = _shard_inputs(x, Wq, bq, Wk, bk, Wv, bv, Wo, bo)
    res = run_bass_kernel_spmd(nc, in_maps, core_ids=list(range(NCORES)))
    kernel._last_results = res

    acc = np.zeros((HIDDEN, NTOK), dtype=np.float32)
    for r in res.results:
        acc += np.asarray(r["outT"]).astype(np.float32)
    out = acc.T.reshape(B, S, HIDDEN)
    out += (bo + bv @ Wo.T).astype(np.float32)
    return out.astype(np.float32)
